# revision 34
# baseline (speedup 1.0000x reference)
"""GAT layer (PyG GATConv, concat=False, edge_dim=1) on 8 Trainium2 cores.

Sharding: core c owns destination nodes [1280c, 1280(c+1)) (last core 1040),
for ALL 4 batches. The graph is batch-independent, so the per-edge gather row
carries all 4 batches' source features at once, and the edge bookkeeping
(indicator matrices, descriptors) is shared across batches -- 4x less
descriptor-generation and indicator work than a (batch x range) split.

Per core:
  phase 1: h[b] = x[b] @ [W | Wa_src | Wa_dst] for all N nodes, 4 batches.
    tableA row (node n, bf16, 2304 B): 4 x [h_b (256 bf16) | a_src_b (4 f32,
    stored as 8 bf16 slots via bitcast) | pad] -> gathered per edge by src.
    tableB row (node n, f32 64 els): a_dst for 4 batches (16 f32) -> gathered
    once per destination tile (not per edge).
  phase 2: edges sorted by dst, 128-edge blocks per 128-node dst tile
    (block counts shared across cores = max, so one SPMD program).
    Per chunk (8 blocks): dma_gather source rows; alpha = a_src + attr*c
    (+ a_dst via IndT matmul from SBUF); leakyrelu via max(x, 0.2x);
    p = exp on ScalarE, broadcast-expanded to [b,h,o].
    Per block: Ind[e,n] = (rel_dst[e]==n) via iota+is_equal; PSUM
    accumulation accn += Ind.T @ (p * h_src)  (numerator, 1024 cols) and
    accd += Ind.T @ p (softmax denominator, 16 cols).  No max-subtraction:
    |alpha| <= ~10 here so exp is safe in f32, softmax unchanged.
  epilogue per tile: divide, mean over heads, + bias.
"""

import numpy as np
import ml_dtypes

B, N, E, D, H, O = 4, 10000, 160000, 128, 4, 64
NEG_SLOPE = 0.2
P = 128
HO = H * O                        # 256
NPC = 1280                        # dst nodes per core
NT = NPC // P                     # 10 dst tiles per core
N_NT = -(-N // P)                 # 79 node tiles for h build
NROWT = N_NT * P                  # 10112 table rows
RB = 272                          # bf16 elems per batch seg: 256 h + 8 + 8
ROW_A = 1152                      # bf16 elems per tableA row (2304 B)
ROW_B = 128                       # bf16 els per tableB row (256 B)
FW = B * HO                       # 1024: phg width
BH = B * H                        # 16
CHUNK = 8                         # blocks per gather call (1024 edges max)
NCORE = 8
NQ = 1                            # SWDGE queues used (Tile sem lanes are
                                  # queue-agnostic; >1 risks lane/queue clash)
ADST = "indT"                     # "gather": per-edge a_dst rows (Q7-heavy)
                                  # "indT": on-chip expand via transposed
                                  #         indicator matmul (DVE/PE-heavy)

_cache = {}


def _build_program(meta):
    import concourse.bacc as bacc
    import concourse.mybir as mybir
    from concourse.tile import TileContext
    from concourse.library_config import mlp

    f32 = mybir.dt.float32
    bf16 = mybir.dt.bfloat16
    i16 = mybir.dt.int16
    i32 = mybir.dt.int32
    Alu = mybir.AluOpType
    Act = mybir.ActivationFunctionType

    nblk = meta["nblk"]
    blk_tile = meta["blk_tile"]
    blk_first = meta["blk_first"]
    blk_last = meta["blk_last"]
    ne = nblk * P
    nch = ne // (CHUNK * P)

    nc = bacc.Bacc("TRN2", target_bir_lowering=False, debug=False,
                   num_devices=NCORE, num_swdge_queues=4)

    xT = nc.dram_tensor("xT", [B, P, N], bf16, kind="ExternalInput")
    w_ext = nc.dram_tensor("w_ext", [P, HO + 2 * H], bf16,
                           kind="ExternalInput")
    crep = nc.dram_tensor("crep", [P, P], f32, kind="ExternalInput")
    bias_bc = nc.dram_tensor("bias_bc", [P, B * O], f32, kind="ExternalInput")
    attr_s = nc.dram_tensor("attr_s", [P, nblk], f32, kind="ExternalInput")
    indtab = nc.dram_tensor("indtab", [nch, P, CHUNK * P], bf16,
                            kind="ExternalInput")
    if ADST == "indT":
        indTtab = nc.dram_tensor("indTtab", [nch, P, CHUNK * P], bf16,
                                 kind="ExternalInput")
    idxA = nc.dram_tensor("idxA", [P, ne // 16], i16, kind="ExternalInput")
    if ADST == "gather":
        idxB = nc.dram_tensor("idxB", [P, ne // 16], i16, kind="ExternalInput")
    else:
        idxT = nc.dram_tensor("idxT", [P, NT * P // 16], i16,
                              kind="ExternalInput")
    y = nc.dram_tensor("y", [NPC, B * O], f32, kind="ExternalOutput")

    tableA = nc.dram_tensor("tableA", [NROWT, ROW_A], bf16, kind="Internal")
    tableB = nc.dram_tensor("tableB", [NROWT, ROW_B], bf16, kind="Internal")

    with TileContext(nc) as tc:
        with (
            tc.tile_pool(name="persist", bufs=1) as pp,
        ):
            nc.gpsimd.load_library(mlp)

            # persistent small tiles
            crep_sb = pp.tile([P, P], f32)
            nc.sync.dma_start(out=crep_sb[:], in_=crep[:])
            bias_sb = pp.tile([P, B * O], f32)
            nc.sync.dma_start(out=bias_sb[:], in_=bias_bc[:])
            attr_sb = pp.tile([P, nblk], f32)
            nc.sync.dma_start(out=attr_sb[:], in_=attr_s[:])
            idxA_sb = pp.tile([P, ne // 16], i16)
            nc.sync.dma_start(out=idxA_sb[:], in_=idxA[:])
            if ADST == "gather":
                idxB_sb = pp.tile([P, ne // 16], i16)
                nc.sync.dma_start(out=idxB_sb[:], in_=idxB[:])
            else:
                idxT_sb = pp.tile([P, NT * P // 16], i16)
                nc.sync.dma_start(out=idxT_sb[:], in_=idxT[:])


            asd_all = pp.tile([P, N_NT, BH], bf16)
            nc.vector.memset(asd_all[:], 0.0)
            out_sb = pp.tile([P, NT, B * O], f32)

            # ---- phase 1 ----
            # t-outer / b-inner: all 4 xT resident (bf16, 20KB/part each),
            # per-tile tableA row writes stream out as soon as computed.
            with (
                tc.tile_pool(name="p1x", bufs=1) as p1x,
                tc.tile_pool(name="p1h", bufs=4) as p1h,
                tc.tile_pool(name="psum_h", bufs=6, space="PSUM") as psh,
            ):
                wext_sb = p1x.tile([P, HO + 2 * H], bf16, tag="wext")
                nc.sync.dma_start(out=wext_sb[:], in_=w_ext[:])
                xTs = []
                for b in range(B):
                    xT_sb = p1x.tile([P, N], bf16, tag=f"xt{b}")
                    nc.sync.dma_start(out=xT_sb[:], in_=xT.ap()[b])
                    xTs.append(xT_sb)
                WG = 4                       # tiles per tableA write
                hst4 = None
                for t in range(N_NT):
                    m = min(P, N - t * P)
                    tq = t % WG
                    if tq == 0:
                        ng = min(WG, N_NT - t)
                        hst4 = p1h.tile([P, WG, ROW_A], bf16, tag="hst")
                    hst = hst4[:, tq, :]
                    if m < P:
                        nc.vector.memset(hst, 0.0)
                    else:
                        nc.vector.memset(hst[:, B * RB:], 0.0)
                        nc.vector.memset(
                            hst[:, 0:B * RB].rearrange(
                                "p (b c) -> p b c", b=B)[:, :, RB - 8:RB],
                            0.0)
                    hstf = hst.bitcast(f32)           # [P, 576]
                    for b in range(B):
                        hps = psh.tile([P, HO + 2 * H], f32, space="PSUM",
                                       tag="hps")
                        nc.tensor.matmul(hps[:m, :],
                                         lhsT=xTs[b][:, t * P:t * P + m],
                                         rhs=wext_sb[:], start=True, stop=True)
                        # h -> bf16, split ScalarE / VectorE
                        nc.scalar.copy(hst[:m, b * RB:b * RB + HO // 2],
                                       hps[:m, 0:HO // 2])
                        nc.vector.tensor_copy(
                            hst[:m, b * RB + HO // 2:b * RB + HO],
                            hps[:m, HO // 2:HO])
                        # a_src (f32 bits inside the bf16 row)
                        nc.vector.tensor_copy(
                            hstf[:m, (b * RB + HO) // 2:(b * RB + HO) // 2 + H],
                            hps[:m, HO:HO + H])
                        # a_dst staging
                        nc.vector.tensor_copy(
                            asd_all[:m, t, b * H:(b + 1) * H],
                            hps[:m, HO + H:])
                    if tq == ng - 1:
                        t0 = t - tq
                        rows = min(WG * P, N - t0 * P)
                        nc.sync.dma_start(
                            out=tableA.ap()[t0 * P:t0 * P + ng * P, :]
                            .rearrange("(q p) c -> p q c", p=P),
                            in_=hst4[:, 0:ng, :])
                # tableB write (all batches at once)
                nc.sync.dma_start(
                    out=tableB.ap()[:, 0:BH].rearrange("(t p) c -> p t c", p=P),
                    in_=asd_all[:])

            # ---- phase 2 ----
            with (
                tc.tile_pool(name="ga", bufs=3) as gap,
                tc.tile_pool(name="rr", bufs=2) as rrp,
                tc.tile_pool(name="wk", bufs=3) as wp,
                tc.tile_pool(name="bk", bufs=4) as bp,
                tc.tile_pool(name="psum_num", bufs=2, space="PSUM") as psn,
                tc.tile_pool(name="psum_den", bufs=2, space="PSUM") as psd,
                tc.tile_pool(name="psum_t", bufs=2, space="PSUM") as pst,
            ):
                # SWDGE queue must equal (pool-DMA issue index) % NQ so
                # Tile's round-robin DMASW lanes pair consistently w/ queues.
                qctr = [0]

                def nextq():
                    v = qctr[0] % NQ
                    qctr[0] += 1
                    return v

                if ADST == "indT":
                    asd_own = pp.tile([P, NT, ROW_B], bf16)
                    for gi in range(2):
                        nc.gpsimd.dma_gather(
                            asd_own[:, gi * (NT // 2):(gi + 1) * (NT // 2), :],
                            tableB.ap()[:, :],
                            idxT_sb[:, gi * 40:(gi + 1) * 40],
                            NT * P // 2, NT * P // 2, ROW_B,
                            queue_num=nextq(), single_packet=False)

                for ch in range(nch):
                    ga = gap.tile([P, CHUNK, ROW_A], bf16, tag="ga")
                    nc.gpsimd.dma_gather(ga[:], tableA.ap()[:, :],
                                         idxA_sb[:, ch * 64:(ch + 1) * 64],
                                         CHUNK * P, CHUNK * P, ROW_A,
                                         queue_num=nextq(),
                                         single_packet=False)
                    if ADST == "gather":
                        gb = rrp.tile([P, CHUNK, ROW_B], bf16, tag="gb")
                        nc.gpsimd.dma_gather(gb[:], tableB.ap()[:, :],
                                             idxB_sb[:, ch * 64:(ch + 1) * 64],
                                             CHUNK * P, CHUNK * P, ROW_B,
                                             queue_num=nextq(),
                                             single_packet=False)
                    ind_sb = rrp.tile([P, CHUNK * P], bf16, tag="inds")
                    nc.sync.dma_start(out=ind_sb[:], in_=indtab.ap()[ch])
                    if ADST == "indT":
                        indT_sb = rrp.tile([P, CHUNK * P], bf16, tag="indTs")
                        nc.sync.dma_start(out=indT_sb[:], in_=indTtab.ap()[ch])

                    gaf = ga[:].bitcast(f32)      # [P, CHUNK, 576]
                    CW = CHUNK * BH               # 128
                    # alpha = attr*c (+ a_dst) + a_src
                    alc = wp.tile([P, CW], f32, tag="alc")
                    al3 = alc[:].rearrange("p (k c) -> p k c", k=CHUNK)
                    al4 = alc[:].rearrange("p (k b h) -> p k b h", k=CHUNK, b=B)
                    nc.vector.tensor_tensor(
                        al3,
                        attr_sb[:, ch * CHUNK:(ch + 1) * CHUNK]
                        .to_broadcast([P, CHUNK, BH]),
                        crep_sb[:].rearrange("p (k c) -> p k c", k=CHUNK),
                        Alu.mult)
                    if ADST == "gather":
                        nc.vector.tensor_tensor(al3, al3, gb[:, :, 0:BH],
                                                Alu.add)
                    nc.vector.tensor_tensor(
                        al4, al4,
                        gaf[:, :, 0:B * (RB // 2)].rearrange(
                            "p k (b c) -> p k b c", b=B)[:, :, :, HO // 2:
                                                         HO // 2 + H],
                        Alu.add)
                    if ADST == "indT":
                        for b8 in range(CHUNK):
                            blk = ch * CHUNK + b8
                            t = blk_tile[blk]
                            adst = psd.tile([P, BH], f32, space="PSUM",
                                            tag="adst")
                            nc.tensor.matmul(
                                adst[:],
                                lhsT=indT_sb[:, b8 * P:(b8 + 1) * P],
                                rhs=asd_own[:, t, 0:BH],
                                start=True, stop=True)
                            nc.vector.tensor_tensor(
                                alc[:, b8 * BH:(b8 + 1) * BH],
                                alc[:, b8 * BH:(b8 + 1) * BH], adst[:],
                                Alu.add)

                    # leaky relu: max(x, 0.2x)
                    lr = wp.tile([P, CW], f32, tag="lr")
                    nc.vector.tensor_scalar(lr[:], alc[:], NEG_SLOPE, None,
                                            Alu.mult)
                    nc.vector.tensor_tensor(lr[:], lr[:], alc[:], Alu.max)
                    # p = exp, expanded to [blk, bh, o] in ONE ScalarE op
                    pxc = wp.tile([P, CHUNK * FW], bf16, tag="pxc")
                    nc.scalar.activation(
                        pxc[:].rearrange("p (c o) -> p c o", c=CHUNK * BH),
                        lr[:].to_broadcast([P, CHUNK * BH, O]),
                        Act.Exp)

                    for b8 in range(CHUNK):
                        blk = ch * CHUNK + b8
                        t = blk_tile[blk]
                        px = pxc[:, b8 * FW:(b8 + 1) * FW]
                        # phg = p * h_src
                        phg = bp.tile([P, FW], bf16, tag="phg")
                        nc.vector.tensor_tensor(
                            phg[:].rearrange("p (b h o) -> p b h o", b=B, h=H),
                            ga[:, b8, 0:B * RB].rearrange(
                                "p (b c) -> p b c", b=B)[:, :, 0:HO]
                            .rearrange("p b (h o) -> p b h o", h=H),
                            px.rearrange("p (b h o) -> p b h o", b=B, h=H),
                            Alu.mult)
                        if blk_first[blk]:
                            accn = psn.tile([P, FW], f32, space="PSUM",
                                            tag="an")
                            accd = pst.tile([P, BH], f32, space="PSUM",
                                            tag="ad")
                            meta["psum_tiles"][t] = (accn, accd)
                        accn, accd = meta["psum_tiles"][t]
                        nc.tensor.matmul(accn[:, 0:FW // 2],
                                         lhsT=ind_sb[:, b8 * P:(b8 + 1) * P],
                                         rhs=phg[:, 0:FW // 2],
                                         start=blk_first[blk],
                                         stop=blk_last[blk],
                                         skip_group_check=True)
                        nc.tensor.matmul(accn[:, FW // 2:],
                                         lhsT=ind_sb[:, b8 * P:(b8 + 1) * P],
                                         rhs=phg[:, FW // 2:],
                                         start=blk_first[blk],
                                         stop=blk_last[blk],
                                         skip_group_check=True)
                        nc.tensor.matmul(
                            accd[:], lhsT=ind_sb[:, b8 * P:(b8 + 1) * P],
                            rhs=pxc[:, b8 * FW:(b8 + 1) * FW:O],
                            start=blk_first[blk], stop=blk_last[blk],
                            skip_group_check=True)

                        if blk_last[blk]:
                            den = bp.tile([P, BH], f32, tag="den")
                            nc.vector.tensor_scalar(den[:], accd[:], 1e-16,
                                                    None, Alu.max)
                            rec = bp.tile([P, BH], f32, tag="rec")
                            nc.vector.reciprocal(rec[:], den[:])
                            onum = bp.tile([P, FW], f32, tag="onum")
                            nc.vector.tensor_tensor(
                                onum[:].rearrange("p (c o) -> p c o", c=BH),
                                accn[:].rearrange("p (c o) -> p c o", c=BH),
                                rec[:].to_broadcast([P, BH, O]), Alu.mult)
                            hsum = bp.tile([P, B * O], f32, tag="hsum")
                            nc.vector.tensor_reduce(
                                hsum[:].rearrange("p (b o) -> p b o", b=B),
                                onum[:].rearrange("p (b h o) -> p b o h",
                                                  b=B, h=H),
                                axis=mybir.AxisListType.X, op=Alu.add)
                            nc.vector.tensor_scalar(hsum[:], hsum[:], 1.0 / H,
                                                    None, Alu.mult)
                            nc.vector.tensor_tensor(out_sb[:, t, :], hsum[:],
                                                    bias_sb[:], Alu.add)

                # final output
                nc.sync.dma_start(
                    out=y.ap().rearrange("(t p) o -> p t o", p=P),
                    in_=out_sb[:])

    nc.compile()
    return nc


def _preprocess(inputs):
    x = np.asarray(inputs["x"], np.float32)
    edge_index = np.asarray(inputs["edge_index"])
    edge_attr = np.asarray(inputs["edge_attr"], np.float32)
    W_src = np.asarray(inputs["W_src"], np.float32)
    att_src = np.asarray(inputs["att_src"], np.float32)
    att_dst = np.asarray(inputs["att_dst"], np.float32)
    W_edge = np.asarray(inputs["W_edge"], np.float32)
    att_edge = np.asarray(inputs["att_edge"], np.float32)
    bias = np.asarray(inputs["bias"], np.float32)

    src = edge_index[0].astype(np.int64)
    dst = edge_index[1].astype(np.int64)

    W_flat = W_src.reshape(D, HO)
    Wa_src = np.einsum("dho,ho->dh", W_src, att_src)
    Wa_dst = np.einsum("dho,ho->dh", W_src, att_dst)
    w_ext = np.ascontiguousarray(
        np.concatenate([W_flat, Wa_src, Wa_dst], axis=1))
    c = np.einsum("ho,ho->h", W_edge, att_edge)              # [4]
    # crep[p, 16k + 4b + h] = c[h]
    crep = np.tile(np.tile(c, B), CHUNK)[None, :].repeat(P, 0).copy()
    bias_bc = np.tile(bias, B)[None, :].repeat(P, 0).copy()

    # per-core dst ranges
    per_core = []
    cnt = np.zeros((NCORE, NT), np.int64)
    for core in range(NCORE):
        lo, hi = core * NPC, min((core + 1) * NPC, N)
        sel = np.nonzero((dst >= lo) & (dst < hi))[0]
        ld = dst[sel] - lo
        order = np.argsort(ld, kind="stable")
        sel, ld = sel[order], ld[order]
        tiles = ld // P
        cnt[core] = np.bincount(tiles, minlength=NT)
        per_core.append((sel, ld, tiles))

    bt = np.maximum(1, -(-cnt.max(axis=0) // P))
    total = int(bt.sum())
    bt[NT - 1] += -(-total // CHUNK) * CHUNK - total
    nblk = int(bt.sum())
    ne = nblk * P
    starts = np.concatenate([[0], np.cumsum(bt)])

    blk_tile = np.repeat(np.arange(NT), bt)
    blk_first = np.zeros(nblk, bool)
    blk_last = np.zeros(nblk, bool)
    blk_first[starts[:-1]] = True
    blk_last[starts[1:] - 1] = True

    meta = {"nblk": nblk, "blk_tile": blk_tile.tolist(),
            "blk_first": blk_first.tolist(), "blk_last": blk_last.tolist(),
            "psum_tiles": {}}

    def wrap16(a, chunklen=1024):
        # idx j of each chunklen-call -> partition j%16, col j//16; x8 replicate
        ncalls = len(a) // chunklen
        w = a.astype(np.int16).reshape(ncalls, chunklen // 16, 16)
        w = w.transpose(2, 0, 1).reshape(16, -1)
        return np.tile(w, (8, 1)).copy()

    in_maps = []
    for core in range(NCORE):
        sel, ld, tiles = per_core[core]
        srcg = np.zeros(ne, np.int64)
        dstg = np.zeros(ne, np.int64)
        attr = np.zeros(ne, np.float32)
        reld = np.full(ne, -1.0, np.float32)
        tcnt = np.bincount(tiles, minlength=NT)
        ofs = np.arange(len(sel)) - np.repeat(
            np.concatenate([[0], np.cumsum(tcnt)])[:-1], tcnt)
        slot = starts[tiles] * P + ofs
        srcg[slot] = src[sel]
        dstg[slot] = dst[sel]
        attr[slot] = edge_attr[sel]
        reld[slot] = (ld - tiles * P).astype(np.float32)

        nch = ne // 1024
        # indicator tables: ind[e, n] = (rel_dst[e] == n), and its per-block
        # transpose; laid out so each chunk is one contiguous [128, 1024] DMA
        rel_b = reld.reshape(nblk, P)                       # [blk, e]
        ind_full = (rel_b[:, :, None] ==
                    np.arange(P)[None, None, :])            # [blk, e, n]
        indtab = np.ascontiguousarray(
            ind_full.transpose(1, 0, 2).reshape(P, nblk, P)
            .reshape(P, nch, CHUNK * P).transpose(1, 0, 2)
        ).astype(ml_dtypes.bfloat16)
        indT_full = ind_full.transpose(0, 2, 1)             # [blk, n, e]
        indTtab = np.ascontiguousarray(
            indT_full.transpose(1, 0, 2).reshape(P, nblk, P)
            .reshape(P, nch, CHUNK * P).transpose(1, 0, 2)
        ).astype(ml_dtypes.bfloat16)
        m = {
            "idxA": wrap16(srcg),
            "attr_s": np.ascontiguousarray(attr.reshape(nblk, P).T),
            "indtab": indtab,
            "xT": np.ascontiguousarray(
                x.transpose(0, 2, 1)).astype(ml_dtypes.bfloat16),
            "w_ext": w_ext.astype(ml_dtypes.bfloat16),
            "crep": crep.astype(np.float32),
            "bias_bc": bias_bc.astype(np.float32),
        }
        if ADST == "gather":
            m["idxB"] = wrap16(dstg)
        else:
            own = (np.arange(NT * P) + core * NPC).clip(max=N - 1)
            m["idxT"] = wrap16(own, chunklen=640)
            m["indTtab"] = indTtab
        in_maps.append(m)
    return meta, in_maps


def kernel(**inputs):
    from concourse.bass_utils import run_bass_kernel_spmd

    meta, in_maps = _preprocess(inputs)
    key = meta["nblk"]
    if key not in _cache:
        _cache[key] = _build_program(meta)
    nc = _cache[key]

    res = run_bass_kernel_spmd(nc, in_maps, core_ids=list(range(NCORE)))
    out = np.empty((B, N, O), np.float32)
    for core in range(NCORE):
        lo, hi = core * NPC, min((core + 1) * NPC, N)
        yc = res.results[core]["y"]                 # [1280, 256]
        for b in range(B):
            out[b, lo:hi, :] = yc[:hi - lo, b * O:(b + 1) * O]
    return out


# revision 35
# speedup vs baseline: 1.0339x; 1.0339x over previous
"""GAT layer (PyG GATConv, concat=False, edge_dim=1) on 8 Trainium2 cores.

Sharding: core c owns destination nodes [1280c, 1280(c+1)) (last core 1040),
for ALL 4 batches. The graph is batch-independent, so the per-edge gather row
carries all 4 batches' source features at once, and the edge bookkeeping
(indicator matrices, descriptors) is shared across batches -- 4x less
descriptor-generation and indicator work than a (batch x range) split.

Per core:
  phase 1: h[b] = x[b] @ [W | Wa_src | Wa_dst] for all N nodes, 4 batches.
    tableA row (node n, bf16, 2304 B): 4 x [h_b (256 bf16) | a_src_b (4 f32,
    stored as 8 bf16 slots via bitcast) | pad] -> gathered per edge by src.
    tableB row (node n, f32 64 els): a_dst for 4 batches (16 f32) -> gathered
    once per destination tile (not per edge).
  phase 2: edges sorted by dst, 128-edge blocks per 128-node dst tile
    (block counts shared across cores = max, so one SPMD program).
    Per chunk (8 blocks): dma_gather source rows; alpha = a_src + attr*c
    (+ a_dst via IndT matmul from SBUF); leakyrelu via max(x, 0.2x);
    p = exp on ScalarE, broadcast-expanded to [b,h,o].
    Per block: Ind[e,n] = (rel_dst[e]==n) via iota+is_equal; PSUM
    accumulation accn += Ind.T @ (p * h_src)  (numerator, 1024 cols) and
    accd += Ind.T @ p (softmax denominator, 16 cols).  No max-subtraction:
    |alpha| <= ~10 here so exp is safe in f32, softmax unchanged.
  epilogue per tile: divide, mean over heads, + bias.
"""

import numpy as np
import ml_dtypes

B, N, E, D, H, O = 4, 10000, 160000, 128, 4, 64
NEG_SLOPE = 0.2
P = 128
HO = H * O                        # 256
NPC = 1280                        # dst nodes per core
NT = NPC // P                     # 10 dst tiles per core
N_NT = -(-N // P)                 # 79 node tiles for h build
NROWT = N_NT * P                  # 10112 table rows
RB = 272                          # bf16 elems per batch seg: 256 h + 8 + 8
ROW_A = 1152                      # bf16 elems per tableA row (2304 B)
ROW_B = 128                       # bf16 els per tableB row (256 B)
FW = B * HO                       # 1024: phg width
BH = B * H                        # 16
CHUNK = 8                         # blocks per gather call (1024 edges max)
NCORE = 8
NQ = 1                            # SWDGE queues used (Tile sem lanes are
                                  # queue-agnostic; >1 risks lane/queue clash)
ADST = "indT"                     # "gather": per-edge a_dst rows (Q7-heavy)
                                  # "indT": on-chip expand via transposed
                                  #         indicator matmul (DVE/PE-heavy)

_cache = {}


def _build_program(meta):
    import concourse.bacc as bacc
    import concourse.mybir as mybir
    from concourse.tile import TileContext
    from concourse.library_config import mlp

    f32 = mybir.dt.float32
    bf16 = mybir.dt.bfloat16
    i16 = mybir.dt.int16
    i32 = mybir.dt.int32
    Alu = mybir.AluOpType
    Act = mybir.ActivationFunctionType

    nblk = meta["nblk"]
    blk_tile = meta["blk_tile"]
    blk_first = meta["blk_first"]
    blk_last = meta["blk_last"]
    ne = nblk * P
    nch = ne // (CHUNK * P)

    nc = bacc.Bacc("TRN2", target_bir_lowering=False, debug=False,
                   num_devices=NCORE, num_swdge_queues=4)

    xT = nc.dram_tensor("xT", [B, P, N], bf16, kind="ExternalInput")
    w_ext = nc.dram_tensor("w_ext", [P, HO + 2 * H], bf16,
                           kind="ExternalInput")
    crep = nc.dram_tensor("crep", [P, P], f32, kind="ExternalInput")
    bias_bc = nc.dram_tensor("bias_bc", [P, B * O], f32, kind="ExternalInput")
    attr_s = nc.dram_tensor("attr_s", [P, nblk], f32, kind="ExternalInput")
    indtab = nc.dram_tensor("indtab", [nch, P, CHUNK * P], bf16,
                            kind="ExternalInput")
    if ADST == "indT":
        indTtab = nc.dram_tensor("indTtab", [nch, P, CHUNK * P], bf16,
                                 kind="ExternalInput")
    idxA = nc.dram_tensor("idxA", [P, ne // 16], i16, kind="ExternalInput")
    if ADST == "gather":
        idxB = nc.dram_tensor("idxB", [P, ne // 16], i16, kind="ExternalInput")
    else:
        idxT = nc.dram_tensor("idxT", [P, NT * P // 16], i16,
                              kind="ExternalInput")
    y = nc.dram_tensor("y", [NPC, B * O], f32, kind="ExternalOutput")

    tableA = nc.dram_tensor("tableA", [NROWT, ROW_A], bf16, kind="Internal")
    tableB = nc.dram_tensor("tableB", [NROWT, ROW_B], bf16, kind="Internal")

    with TileContext(nc) as tc:
        with (
            tc.tile_pool(name="persist", bufs=1) as pp,
        ):
            nc.gpsimd.load_library(mlp)

            # persistent small tiles
            crep_sb = pp.tile([P, P], f32)
            nc.sync.dma_start(out=crep_sb[:], in_=crep[:])
            bias_sb = pp.tile([P, B * O], f32)
            nc.sync.dma_start(out=bias_sb[:], in_=bias_bc[:])
            attr_sb = pp.tile([P, nblk], f32)
            nc.sync.dma_start(out=attr_sb[:], in_=attr_s[:])
            idxA_sb = pp.tile([P, ne // 16], i16)
            nc.sync.dma_start(out=idxA_sb[:], in_=idxA[:])
            if ADST == "gather":
                idxB_sb = pp.tile([P, ne // 16], i16)
                nc.sync.dma_start(out=idxB_sb[:], in_=idxB[:])
            else:
                idxT_sb = pp.tile([P, NT * P // 16], i16)
                nc.sync.dma_start(out=idxT_sb[:], in_=idxT[:])


            asd_all = pp.tile([P, N_NT, BH], bf16)
            nc.vector.memset(asd_all[:], 0.0)
            out_sb = pp.tile([P, NT, B * O], f32)

            # ---- phase 1 ----
            # t-outer / b-inner: all 4 xT resident (bf16, 20KB/part each),
            # per-tile tableA row writes stream out as soon as computed.
            with (
                tc.tile_pool(name="p1x", bufs=1) as p1x,
                tc.tile_pool(name="p1h", bufs=4) as p1h,
                tc.tile_pool(name="psum_h", bufs=6, space="PSUM") as psh,
            ):
                wext_sb = p1x.tile([P, HO + 2 * H], bf16, tag="wext")
                nc.sync.dma_start(out=wext_sb[:], in_=w_ext[:])
                xTs = []
                for b in range(B):
                    xT_sb = p1x.tile([P, N], bf16, tag=f"xt{b}")
                    nc.sync.dma_start(out=xT_sb[:], in_=xT.ap()[b])
                    xTs.append(xT_sb)
                WG = 4                       # tiles per tableA write
                hst4 = None
                for t in range(N_NT):
                    m = min(P, N - t * P)
                    tq = t % WG
                    if tq == 0:
                        ng = min(WG, N_NT - t)
                        hst4 = p1h.tile([P, WG, ROW_A], bf16, tag="hst")
                    hst = hst4[:, tq, :]
                    if m < P:
                        nc.vector.memset(hst, 0.0)
                    else:
                        nc.vector.memset(hst[:, B * RB:], 0.0)
                        nc.vector.memset(
                            hst[:, 0:B * RB].rearrange(
                                "p (b c) -> p b c", b=B)[:, :, RB - 8:RB],
                            0.0)
                    hstf = hst.bitcast(f32)           # [P, 576]
                    for b in range(B):
                        hps = psh.tile([P, HO + 2 * H], f32, space="PSUM",
                                       tag="hps")
                        nc.tensor.matmul(hps[:m, :],
                                         lhsT=xTs[b][:, t * P:t * P + m],
                                         rhs=wext_sb[:], start=True, stop=True)
                        # h -> bf16, split ScalarE / VectorE
                        nc.scalar.copy(hst[:m, b * RB:b * RB + HO // 2],
                                       hps[:m, 0:HO // 2])
                        nc.vector.tensor_copy(
                            hst[:m, b * RB + HO // 2:b * RB + HO],
                            hps[:m, HO // 2:HO])
                        # a_src (f32 bits inside the bf16 row)
                        nc.vector.tensor_copy(
                            hstf[:m, (b * RB + HO) // 2:(b * RB + HO) // 2 + H],
                            hps[:m, HO:HO + H])
                        # a_dst staging
                        nc.vector.tensor_copy(
                            asd_all[:m, t, b * H:(b + 1) * H],
                            hps[:m, HO + H:])
                    if tq == ng - 1:
                        t0 = t - tq
                        rows = min(WG * P, N - t0 * P)
                        nc.sync.dma_start(
                            out=tableA.ap()[t0 * P:t0 * P + ng * P, :]
                            .rearrange("(q p) c -> p q c", p=P),
                            in_=hst4[:, 0:ng, :])
                # tableB write (all batches at once)
                nc.sync.dma_start(
                    out=tableB.ap()[:, 0:BH].rearrange("(t p) c -> p t c", p=P),
                    in_=asd_all[:])

            # ---- phase 2 ----
            with (
                tc.tile_pool(name="ga", bufs=3) as gap,
                tc.tile_pool(name="rr", bufs=2) as rrp,
                tc.tile_pool(name="wk", bufs=3) as wp,
                tc.tile_pool(name="bk", bufs=4) as bp,
                tc.tile_pool(name="psum_num", bufs=2, space="PSUM") as psn,
                tc.tile_pool(name="psum_den", bufs=2, space="PSUM") as psd,
                tc.tile_pool(name="psum_t", bufs=2, space="PSUM") as pst,
            ):
                # SWDGE queue must equal (pool-DMA issue index) % NQ so
                # Tile's round-robin DMASW lanes pair consistently w/ queues.
                qctr = [0]

                def nextq():
                    v = qctr[0] % NQ
                    qctr[0] += 1
                    return v

                if ADST == "indT":
                    asd_own = pp.tile([P, NT, ROW_B], bf16)
                    for gi in range(2):
                        nc.gpsimd.dma_gather(
                            asd_own[:, gi * (NT // 2):(gi + 1) * (NT // 2), :],
                            tableB.ap()[:, :],
                            idxT_sb[:, gi * 40:(gi + 1) * 40],
                            NT * P // 2, NT * P // 2, ROW_B,
                            queue_num=nextq(), single_packet=False)

                for ch in range(nch):
                    ga = gap.tile([P, CHUNK, ROW_A], bf16, tag="ga")
                    nc.gpsimd.dma_gather(ga[:], tableA.ap()[:, :],
                                         idxA_sb[:, ch * 64:(ch + 1) * 64],
                                         CHUNK * P, CHUNK * P, ROW_A,
                                         queue_num=nextq(),
                                         single_packet=False)
                    if ADST == "gather":
                        gb = rrp.tile([P, CHUNK, ROW_B], bf16, tag="gb")
                        nc.gpsimd.dma_gather(gb[:], tableB.ap()[:, :],
                                             idxB_sb[:, ch * 64:(ch + 1) * 64],
                                             CHUNK * P, CHUNK * P, ROW_B,
                                             queue_num=nextq(),
                                             single_packet=False)
                    ind_sb = rrp.tile([P, CHUNK * P], bf16, tag="inds")
                    nc.sync.dma_start(out=ind_sb[:], in_=indtab.ap()[ch])
                    if ADST == "indT":
                        indT_sb = rrp.tile([P, CHUNK * P], bf16, tag="indTs")
                        nc.sync.dma_start(out=indT_sb[:], in_=indTtab.ap()[ch])

                    gaf = ga[:].bitcast(f32)      # [P, CHUNK, 576]
                    CW = CHUNK * BH               # 128
                    # alpha = attr*c (+ a_dst) + a_src
                    alc = wp.tile([P, CW], f32, tag="alc")
                    al3 = alc[:].rearrange("p (k c) -> p k c", k=CHUNK)
                    al4 = alc[:].rearrange("p (k b h) -> p k b h", k=CHUNK, b=B)
                    nc.vector.tensor_tensor(
                        al3,
                        attr_sb[:, ch * CHUNK:(ch + 1) * CHUNK]
                        .to_broadcast([P, CHUNK, BH]),
                        crep_sb[:].rearrange("p (k c) -> p k c", k=CHUNK),
                        Alu.mult)
                    if ADST == "gather":
                        nc.vector.tensor_tensor(al3, al3, gb[:, :, 0:BH],
                                                Alu.add)
                    nc.vector.tensor_tensor(
                        al4, al4,
                        gaf[:, :, 0:B * (RB // 2)].rearrange(
                            "p k (b c) -> p k b c", b=B)[:, :, :, HO // 2:
                                                         HO // 2 + H],
                        Alu.add)
                    if ADST == "indT":
                        for b8 in range(CHUNK):
                            blk = ch * CHUNK + b8
                            t = blk_tile[blk]
                            adst = psd.tile([P, BH], f32, space="PSUM",
                                            tag="adst")
                            nc.tensor.matmul(
                                adst[:],
                                lhsT=indT_sb[:, b8 * P:(b8 + 1) * P],
                                rhs=asd_own[:, t, 0:BH],
                                start=True, stop=True)
                            nc.vector.tensor_tensor(
                                alc[:, b8 * BH:(b8 + 1) * BH],
                                alc[:, b8 * BH:(b8 + 1) * BH], adst[:],
                                Alu.add)

                    # leaky relu: max(x, 0.2x)
                    lr = wp.tile([P, CW], f32, tag="lr")
                    nc.vector.tensor_scalar(lr[:], alc[:], NEG_SLOPE, None,
                                            Alu.mult)
                    nc.vector.tensor_tensor(lr[:], lr[:], alc[:], Alu.max)
                    for b8 in range(CHUNK):
                        blk = ch * CHUNK + b8
                        t = blk_tile[blk]
                        pxt = bp.tile([P, FW], bf16, tag="px")
                        nc.scalar.activation(
                            pxt[:].rearrange("p (c o) -> p c o", c=BH),
                            lr[:, b8 * BH:(b8 + 1) * BH]
                            .to_broadcast([P, BH, O]),
                            Act.Exp)
                        px = pxt[:]
                        # phg = p * h_src
                        phg = bp.tile([P, FW], bf16, tag="phg")
                        nc.vector.tensor_tensor(
                            phg[:].rearrange("p (b h o) -> p b h o", b=B, h=H),
                            ga[:, b8, 0:B * RB].rearrange(
                                "p (b c) -> p b c", b=B)[:, :, 0:HO]
                            .rearrange("p b (h o) -> p b h o", h=H),
                            px.rearrange("p (b h o) -> p b h o", b=B, h=H),
                            Alu.mult)
                        if blk_first[blk]:
                            accn = psn.tile([P, FW], f32, space="PSUM",
                                            tag="an")
                            accd = pst.tile([P, BH], f32, space="PSUM",
                                            tag="ad")
                            meta["psum_tiles"][t] = (accn, accd)
                        accn, accd = meta["psum_tiles"][t]
                        nc.tensor.matmul(accn[:, 0:FW // 2],
                                         lhsT=ind_sb[:, b8 * P:(b8 + 1) * P],
                                         rhs=phg[:, 0:FW // 2],
                                         start=blk_first[blk],
                                         stop=blk_last[blk],
                                         skip_group_check=True)
                        nc.tensor.matmul(accn[:, FW // 2:],
                                         lhsT=ind_sb[:, b8 * P:(b8 + 1) * P],
                                         rhs=phg[:, FW // 2:],
                                         start=blk_first[blk],
                                         stop=blk_last[blk],
                                         skip_group_check=True)
                        nc.tensor.matmul(
                            accd[:], lhsT=ind_sb[:, b8 * P:(b8 + 1) * P],
                            rhs=pxt[:, 0:FW:O],
                            start=blk_first[blk], stop=blk_last[blk],
                            skip_group_check=True)

                        if blk_last[blk]:
                            den = bp.tile([P, BH], f32, tag="den")
                            nc.vector.tensor_scalar(den[:], accd[:], 1e-16,
                                                    None, Alu.max)
                            rec = bp.tile([P, BH], f32, tag="rec")
                            nc.vector.reciprocal(rec[:], den[:])
                            onum = bp.tile([P, FW], f32, tag="onum")
                            nc.vector.tensor_tensor(
                                onum[:].rearrange("p (c o) -> p c o", c=BH),
                                accn[:].rearrange("p (c o) -> p c o", c=BH),
                                rec[:].to_broadcast([P, BH, O]), Alu.mult)
                            hsum = bp.tile([P, B * O], f32, tag="hsum")
                            nc.vector.tensor_reduce(
                                hsum[:].rearrange("p (b o) -> p b o", b=B),
                                onum[:].rearrange("p (b h o) -> p b o h",
                                                  b=B, h=H),
                                axis=mybir.AxisListType.X, op=Alu.add)
                            nc.vector.tensor_scalar(hsum[:], hsum[:], 1.0 / H,
                                                    None, Alu.mult)
                            nc.vector.tensor_tensor(out_sb[:, t, :], hsum[:],
                                                    bias_sb[:], Alu.add)

                # final output
                nc.sync.dma_start(
                    out=y.ap().rearrange("(t p) o -> p t o", p=P),
                    in_=out_sb[:])

    nc.compile()
    return nc


def _preprocess(inputs):
    x = np.asarray(inputs["x"], np.float32)
    edge_index = np.asarray(inputs["edge_index"])
    edge_attr = np.asarray(inputs["edge_attr"], np.float32)
    W_src = np.asarray(inputs["W_src"], np.float32)
    att_src = np.asarray(inputs["att_src"], np.float32)
    att_dst = np.asarray(inputs["att_dst"], np.float32)
    W_edge = np.asarray(inputs["W_edge"], np.float32)
    att_edge = np.asarray(inputs["att_edge"], np.float32)
    bias = np.asarray(inputs["bias"], np.float32)

    src = edge_index[0].astype(np.int64)
    dst = edge_index[1].astype(np.int64)

    W_flat = W_src.reshape(D, HO)
    Wa_src = np.einsum("dho,ho->dh", W_src, att_src)
    Wa_dst = np.einsum("dho,ho->dh", W_src, att_dst)
    w_ext = np.ascontiguousarray(
        np.concatenate([W_flat, Wa_src, Wa_dst], axis=1))
    c = np.einsum("ho,ho->h", W_edge, att_edge)              # [4]
    # crep[p, 16k + 4b + h] = c[h]
    crep = np.tile(np.tile(c, B), CHUNK)[None, :].repeat(P, 0).copy()
    bias_bc = np.tile(bias, B)[None, :].repeat(P, 0).copy()

    # per-core dst ranges
    per_core = []
    cnt = np.zeros((NCORE, NT), np.int64)
    for core in range(NCORE):
        lo, hi = core * NPC, min((core + 1) * NPC, N)
        sel = np.nonzero((dst >= lo) & (dst < hi))[0]
        ld = dst[sel] - lo
        order = np.argsort(ld, kind="stable")
        sel, ld = sel[order], ld[order]
        tiles = ld // P
        cnt[core] = np.bincount(tiles, minlength=NT)
        per_core.append((sel, ld, tiles))

    bt = np.maximum(1, -(-cnt.max(axis=0) // P))
    total = int(bt.sum())
    bt[NT - 1] += -(-total // CHUNK) * CHUNK - total
    nblk = int(bt.sum())
    ne = nblk * P
    starts = np.concatenate([[0], np.cumsum(bt)])

    blk_tile = np.repeat(np.arange(NT), bt)
    blk_first = np.zeros(nblk, bool)
    blk_last = np.zeros(nblk, bool)
    blk_first[starts[:-1]] = True
    blk_last[starts[1:] - 1] = True

    meta = {"nblk": nblk, "blk_tile": blk_tile.tolist(),
            "blk_first": blk_first.tolist(), "blk_last": blk_last.tolist(),
            "psum_tiles": {}}

    def wrap16(a, chunklen=1024):
        # idx j of each chunklen-call -> partition j%16, col j//16; x8 replicate
        ncalls = len(a) // chunklen
        w = a.astype(np.int16).reshape(ncalls, chunklen // 16, 16)
        w = w.transpose(2, 0, 1).reshape(16, -1)
        return np.tile(w, (8, 1)).copy()

    in_maps = []
    for core in range(NCORE):
        sel, ld, tiles = per_core[core]
        srcg = np.zeros(ne, np.int64)
        dstg = np.zeros(ne, np.int64)
        attr = np.zeros(ne, np.float32)
        reld = np.full(ne, -1.0, np.float32)
        tcnt = np.bincount(tiles, minlength=NT)
        ofs = np.arange(len(sel)) - np.repeat(
            np.concatenate([[0], np.cumsum(tcnt)])[:-1], tcnt)
        slot = starts[tiles] * P + ofs
        srcg[slot] = src[sel]
        dstg[slot] = dst[sel]
        attr[slot] = edge_attr[sel]
        reld[slot] = (ld - tiles * P).astype(np.float32)

        nch = ne // 1024
        # indicator tables: ind[e, n] = (rel_dst[e] == n), and its per-block
        # transpose; laid out so each chunk is one contiguous [128, 1024] DMA
        rel_b = reld.reshape(nblk, P)                       # [blk, e]
        ind_full = (rel_b[:, :, None] ==
                    np.arange(P)[None, None, :])            # [blk, e, n]
        indtab = np.ascontiguousarray(
            ind_full.transpose(1, 0, 2).reshape(P, nblk, P)
            .reshape(P, nch, CHUNK * P).transpose(1, 0, 2)
        ).astype(ml_dtypes.bfloat16)
        indT_full = ind_full.transpose(0, 2, 1)             # [blk, n, e]
        indTtab = np.ascontiguousarray(
            indT_full.transpose(1, 0, 2).reshape(P, nblk, P)
            .reshape(P, nch, CHUNK * P).transpose(1, 0, 2)
        ).astype(ml_dtypes.bfloat16)
        m = {
            "idxA": wrap16(srcg),
            "attr_s": np.ascontiguousarray(attr.reshape(nblk, P).T),
            "indtab": indtab,
            "xT": np.ascontiguousarray(
                x.transpose(0, 2, 1)).astype(ml_dtypes.bfloat16),
            "w_ext": w_ext.astype(ml_dtypes.bfloat16),
            "crep": crep.astype(np.float32),
            "bias_bc": bias_bc.astype(np.float32),
        }
        if ADST == "gather":
            m["idxB"] = wrap16(dstg)
        else:
            own = (np.arange(NT * P) + core * NPC).clip(max=N - 1)
            m["idxT"] = wrap16(own, chunklen=640)
            m["indTtab"] = indTtab
        in_maps.append(m)
    return meta, in_maps


def kernel(**inputs):
    from concourse.bass_utils import run_bass_kernel_spmd

    meta, in_maps = _preprocess(inputs)
    key = meta["nblk"]
    if key not in _cache:
        _cache[key] = _build_program(meta)
    nc = _cache[key]

    res = run_bass_kernel_spmd(nc, in_maps, core_ids=list(range(NCORE)))
    out = np.empty((B, N, O), np.float32)
    for core in range(NCORE):
        lo, hi = core * NPC, min((core + 1) * NPC, N)
        yc = res.results[core]["y"]                 # [1280, 256]
        for b in range(B):
            out[b, lo:hi, :] = yc[:hi - lo, b * O:(b + 1) * O]
    return out
